# revision 1
# baseline (speedup 1.0000x reference)
"""Trainium2 Bass kernel: per-row bincount (BagOfWords) over 8 NeuronCores.

Problem: inputs int32 [16384, 200], values in [0, 1100); output f32
[16384, 1099] = per-row histogram over token ids 1..1099 (bin 0 dropped).

Strategy (pure data parallel): shard the batch over 8 cores (2048 rows
each). Per core, factorize each token id v = 35*h + l (h in [0,32),
l in [0,35)) and compute the per-row histogram as a tiny per-row matmul
on the PE systolic array:

    psum[th, tl] = sum_j onehot_h(h_j)[th] * onehot_l(l_j)[tl]

with the contraction over token slots on the partition dim (k = 128 + 72).
Digit tensors are transposed to k-major via PE transpose; one-hot
matrices are built in bf16 on the Vector engine with per-digit
tensor_scalar compares (DVE 4x perf mode), th-major over 512 merged
columns (two 128-row tiles x A/B k-halves per op). Matmul emission is
software-pipelined one 256-row pair behind generation so the next
pair's PE transposes are never queued behind the previous pair's 1024
matmuls. Per-row [32, 35] results are packed 4-across-partitions (PE
col-groups) x 8-across-free per PSUM bank, evicted in bulk on the
Scalar engine, and DMA'd to a padded [2048, 1120] output; the host
drops bins 0 and 1100+ and concatenates shards. All arithmetic is
exact (integer-valued bf16/f32).
"""

import numpy as np
import ml_dtypes
from contextlib import ExitStack

import concourse.bass as bass
import concourse.tile as tile
from concourse import bacc, mybir
from concourse.bass_utils import run_bass_kernel_spmd

BF16 = mybir.dt.bfloat16
F32 = mybir.dt.float32
I32 = mybir.dt.int32
AluOp = mybir.AluOpType

N_CORES = 8
FULL_B = 16384
S = 200
NH, NL = 32, 35
V = NH * NL  # 1120 (>= 1100); bins 0 and 1100..1119 dropped on host
KA, KB = 128, 72
RG = 64


def _host_consts():
    ident = np.eye(128, dtype=np.float32)
    return {"ident": np.ascontiguousarray(ident)}


def _emit_pair_mms(nc, ps_tiles, oh3H, oh3L, stage, pair, y):
    for g in range(4):  # four psum groups of RG=64 rows in the pair
        r0 = g * RG
        ps = ps_tiles[g % 2]
        for r in range(RG):
            rr = r0 + r          # row within the 256-row pair
            tile_half = rr // 128
            rloc = rr % 128
            base = 256 * tile_half
            s = r % 4
            q = (r // 4) % 8
            b2 = r // 32
            out_ap = ps[32 * s:32 * s + NH,
                        512 * b2 + NL * q:512 * b2 + NL * q + NL]
            nc.tensor.matmul(out_ap,
                             oh3H[:, base + rloc, :],
                             oh3L[:, base + rloc, :],
                             start=True, stop=False,
                             tile_position=(0, 32 * s))
            nc.tensor.matmul(out_ap,
                             oh3H[0:KB, base + 128 + rloc, :],
                             oh3L[0:KB, base + 128 + rloc, :],
                             start=False, stop=True,
                             tile_position=(0, 32 * s))

        nc.scalar.copy(
            stage[:, 560 * g:560 * (g + 1)].rearrange(
                "p (b c) -> p b c", c=280),
            ps[:].rearrange("p (b c) -> p b c", c=512)[:, :, 0:280])

    E = pair
    for s in range(4):
        src = stage[32 * s:32 * s + NH, :].rearrange(
            "p (i c) -> p i c", c=NL)
        dst = bass.AP(y, (256 * E + s) * V,
                      [[NL, NH], [4 * V, 64], [1, NL]])
        out_eng = (nc.sync, nc.scalar, nc.sync, nc.scalar)[s]
        out_eng.dma_start(dst, src)



def _kernel_body(ctx, tc, y, x, ident_d):
    B = FULL_B // N_CORES
    nc = tc.nc
    T = B // 128

    const_pool = ctx.enter_context(tc.tile_pool(name="const", bufs=1))
    io_pool = ctx.enter_context(tc.tile_pool(name="io", bufs=3))
    dig_pool = ctx.enter_context(tc.tile_pool(name="dig", bufs=2))
    kt_pool = ctx.enter_context(tc.tile_pool(name="kt", bufs=2))
    oh_pool = ctx.enter_context(tc.tile_pool(name="oh", bufs=2))
    tp_psum = ctx.enter_context(tc.tile_pool(name="tp", bufs=2, space="PSUM"))
    mm_psum = ctx.enter_context(tc.tile_pool(name="mm", bufs=1, space="PSUM"))
    stage_pool = ctx.enter_context(tc.tile_pool(name="stage", bufs=2))

    # Load constants once.
    c_id = const_pool.tile([128, 128], F32, tag="c_id")
    nc.sync.dma_start(c_id[:], ident_d.ap())

    # Persistent psum accumulators (2, used alternately). One-time memset
    # zeroes the partition ranges the matmuls never write (19:32, 51:64, ...)
    # so the batched eviction reads defined data.
    ps_tiles = []
    for i in range(2):
        ps = mm_psum.tile([128, 1024], F32, tag=f"ps{i}")
        nc.vector.memset(ps[:], 0.0)
        ps_tiles.append(ps)

    stage = None
    hT = lT = None
    pend = None
    for t in range(T):
        half = t % 2          # position within a 2-tile pair
        pair = t // 2
        # ---- load + digit extraction (row-major [128 rows, 200 seq]) ----
        xa = io_pool.tile([128, S], I32, tag="xa")
        in_eng = nc.sync if t % 2 == 0 else nc.scalar
        in_eng.dma_start(xa[:], x.ap()[t * 128:(t + 1) * 128, :])

        yq = dig_pool.tile([128, S], F32, tag="yq")
        nc.vector.tensor_scalar(yq[:], xa[:], 1.0 / 35.0, 0.5 / 35.0,
                                AluOp.mult, AluOp.add)
        hi = dig_pool.tile([128, S], I32, tag="hi")
        nc.vector.tensor_copy(hi[:], yq[:])
        ov = dig_pool.tile([128, S], F32, tag="ov")
        nc.vector.scalar_tensor_tensor(ov[:], hi[:], 35.0, xa[:],
                                       AluOp.mult, AluOp.is_gt)
        hc = dig_pool.tile([128, S], F32, tag="hc")
        nc.vector.tensor_tensor(hc[:], hi[:], ov[:], AluOp.subtract)
        lc = dig_pool.tile([128, S], F32, tag="lc")
        nc.vector.scalar_tensor_tensor(lc[:], hc[:], -35.0, xa[:],
                                       AluOp.mult, AluOp.add)

        # ---- transpose to k-major; 2-tile pair shares kt tensors ----
        tp = tp_psum.tile([128, 512], F32, tag="tp")
        nc.tensor.transpose(tp[:, 0:128], hc[:, 0:128], c_id[:])
        nc.tensor.transpose(tp[0:KB, 128:256], hc[:, 128:S], c_id[:])
        nc.tensor.transpose(tp[:, 256:384], lc[:, 0:128], c_id[:])
        nc.tensor.transpose(tp[0:KB, 384:512], lc[:, 128:S], c_id[:])

        if half == 0:
            hT = kt_pool.tile([128, 512], BF16, tag="hT")
            lT = kt_pool.tile([128, 512], BF16, tag="lT")
            if pair < 2:
                nc.vector.memset(hT[64:128, 128:256], 0.0)
                nc.vector.memset(lT[64:128, 128:256], 0.0)
                nc.vector.memset(hT[64:128, 384:512], 0.0)
                nc.vector.memset(lT[64:128, 384:512], 0.0)
        o = 256 * half
        nc.scalar.copy(hT[:, o + 0:o + 128], tp[:, 0:128])
        nc.scalar.copy(hT[0:KB, o + 128:o + 256], tp[0:KB, 128:256])
        nc.scalar.copy(lT[:, o + 0:o + 128], tp[:, 256:384])
        nc.scalar.copy(lT[0:KB, o + 128:o + 256], tp[0:KB, 384:512])

        if half == 0:
            stage = stage_pool.tile([128, 4 * 560], F32, tag="stage")
            continue  # second tile of the pair does gen + MMs for both

        # ---- flush PREVIOUS pair's matmuls (after this pair's PE
        # transposes are queued, so they don't stall behind 1024 MMs) ----
        if pend is not None:
            _emit_pair_mms(nc, ps_tiles, *pend)

        # ---- one-hot generation over the merged pair (512 cols/op) ----
        ohH = oh_pool.tile([128, NH * 512], BF16, tag="ohH")
        ohL = oh_pool.tile([128, NL * 512], BF16, tag="ohL")
        for th in range(NH):
            nc.vector.tensor_scalar(ohH[:, th * 512:(th + 1) * 512],
                                    hT[:], float(th), None, AluOp.is_equal)
        for tl in range(NL):
            nc.vector.tensor_scalar(ohL[:, tl * 512:(tl + 1) * 512],
                                    lT[:], float(tl), None, AluOp.is_equal)
        pend = (ohH[:].rearrange("p (c r) -> p r c", c=NH),
                ohL[:].rearrange("p (c r) -> p r c", c=NL),
                stage, pair, y)
    _emit_pair_mms(nc, ps_tiles, *pend)




def _build_program():
    B = FULL_B // N_CORES
    nc = bacc.Bacc("TRN2", target_bir_lowering=False, debug=False,
                   num_devices=N_CORES)
    x = nc.dram_tensor("x", [B, S], I32, kind="ExternalInput")
    ident = nc.dram_tensor("ident", [128, 128], F32, kind="ExternalInput")
    y = nc.dram_tensor("y", [B, V], F32, kind="ExternalOutput")
    with tile.TileContext(nc) as tc:
        with ExitStack() as ctx:
            _kernel_body(ctx, tc, y, x, ident)
    nc.compile()
    return nc


_program_cache = {}


def _get_program():
    if "nc" not in _program_cache:
        _program_cache["nc"] = _build_program()
    return _program_cache["nc"]


def kernel(**inputs) -> np.ndarray:
    B = FULL_B // N_CORES
    x_full = np.ascontiguousarray(np.asarray(inputs["inputs"], dtype=np.int32))
    assert x_full.shape == (FULL_B, S), x_full.shape

    nc = _get_program()
    consts = _host_consts()
    in_maps = []
    for c in range(N_CORES):
        m = {"x": np.ascontiguousarray(x_full[c * B:(c + 1) * B])}
        m.update(consts)
        in_maps.append(m)

    res = run_bass_kernel_spmd(nc, in_maps, core_ids=list(range(N_CORES)))
    ys = [np.asarray(res.results[c]["y"]) for c in range(N_CORES)]
    full = np.concatenate(ys, axis=0)
    return np.ascontiguousarray(full[:, 1:1100].astype(np.float32))



# revision 5
# speedup vs baseline: 1.2333x; 1.2333x over previous
"""Trainium2 Bass kernel: per-row bincount (BagOfWords) over 8 NeuronCores.

Problem: inputs int32 [16384, 200], values in [0, 1100); output f32
[16384, 1099] = per-row histogram over token ids 1..1099 (bin 0 dropped).

Strategy (pure data parallel): shard the batch over 8 cores (2048 rows
each). Per core, factorize each token id v = 32*h + l (h in [0,35),
l in [0,32)) and compute the per-row histogram as a tiny per-row matmul
on the PE systolic array:

    psum[l, h] = sum_j onehot_l(l_j)[l] * onehot_h(h_j)[h]

with the contraction over token slots on the partition dim (k = 128 + 72).
Digits h = x>>5 (int shift + cast) are extracted row-major and PE-transposed
to k-major alongside x (fp16); l = x - 32h is derived k-major. One-hot
matrices are generated in fp16 with per-bin compares split across the
Vector (DVE 4x mode), GPSIMD, and Activation engines; since l-bins (0..31)
and h-bins (0..34) share scalar values, one compare op covers both digit
tensors ([lo | hT] adjacent in SBUF) for bins 0..31. Matmul emission is
software-pipelined one 256-row pair behind generation. Per-row [32, 35]
results are packed 4-across-partitions (PE col-groups) densely per PSUM
tile, evicted in bulk on the Scalar engine as fp16, and DMA'd to a
[2048, 1120] output in (l-major, h) bin order; the host permutes to
v = 32h + l, drops bins 0 and 1100+, and concatenates shards. All
arithmetic is exact (integer-valued fp16/f32).
"""

import numpy as np
import ml_dtypes
from contextlib import ExitStack

import concourse.bass as bass
import concourse.tile as tile
from concourse import bacc, mybir
from concourse.bass_utils import run_bass_kernel_spmd

FP16 = mybir.dt.float16
F32 = mybir.dt.float32
I32 = mybir.dt.int32
AluOp = mybir.AluOpType
ActFn = mybir.ActivationFunctionType

N_CORES = 8
FULL_B = 16384
S = 200
NL, NH = 32, 35          # v = 32*h + l; psum out = [NL partitions, NH free]
V = NL * NH              # 1120 device bins; host drops 0 and 1100..1119
KA, KB = 128, 72

# engine split for the 35 one-hot compare ops per 256-row pair.
# full bins (l and h share the compare, [128, 1024]): c = 0..31
# half bins (h only, [128, 512]): c = 32, 33, 34
DVE_FULL = tuple(range(0, 25))
POOL_FULL = tuple(range(25, 30))
ACT_FULL = tuple(range(30, 32))
DVE_HALF = (32, 33)
ACT_HALF = (34,)
POOL_HALF = ()


def _host_consts():
    identh = np.eye(128, dtype=np.float16)
    # activation bias table: col j = -(ACT bin value), last col = +1.0
    act_bins = [float(c) for c in ACT_FULL] + [float(c) for c in ACT_HALF]
    ab = np.zeros((128, len(act_bins) + 1), dtype=np.float32)
    for j, c in enumerate(act_bins):
        ab[:, j] = -c
    ab[:, -1] = 1.0
    return {"identh": np.ascontiguousarray(identh),
            "actbias": np.ascontiguousarray(ab)}


def _emit_pair_mms(nc, ps_tiles, stage, oh3, pair, y):
    """Matmuls + eviction for one 256-row pair; oh3 = [128, 1024 cols, 35 bins]."""
    for g in range(4):
        ps = ps_tiles[g // 2]              # tile T holds half-pair rows 128T+
        goff = 560 * (g % 2)
        for r in range(64):
            rr = g * 64 + r
            s = r % 4
            q = (r // 4) % 8
            b2 = r // 32
            half = rr // 128
            rloc = rr % 128
            ca = 256 * half + rloc        # chunk A col (k = 0..127)
            cb = ca + 128                  # chunk B col (k = 128..199)
            out_ap = ps[32 * s:32 * s + NL,
                        goff + 280 * b2 + NH * q:goff + 280 * b2 + NH * q + NH]
            nc.tensor.matmul(out_ap,
                             oh3[:, ca, 0:NL],
                             oh3[:, 512 + ca, 0:NH],
                             start=True, stop=False,
                             tile_position=(0, 32 * s))
            nc.tensor.matmul(out_ap,
                             oh3[0:KB, cb, 0:NL],
                             oh3[0:KB, 512 + cb, 0:NH],
                             start=False, stop=True,
                             tile_position=(0, 32 * s))
        if g % 2 == 1:  # half-pair done: evict psum tile g//2
            t = g // 2
            nc.scalar.copy(stage[:, 1120 * t:1120 * (t + 1)],
                           ps_tiles[t][:])

    # 8 output DMAs: one per (s, psum-tile T); dst row = 128T+64G+32b2+4q+s
    # = 128T + s + 4*i with i = 16G+8b2+q matching stage col 35*i+h.
    E = pair
    for s in range(4):
        for t in range(2):
            src = stage[32 * s:32 * s + NL,
                        1120 * t:1120 * (t + 1)].rearrange(
                "p (i h) -> p i h", h=NH)
            dst = bass.AP(y, (256 * E + 128 * t + s) * V,
                          [[NH, NL], [4 * V, 32], [1, NH]])
            out_eng = (nc.sync, nc.scalar)[(2 * s + t) % 2]
            out_eng.dma_start(dst, src)


def _kernel_body(ctx, tc, y, x, identh_d, actbias_d):
    B = FULL_B // N_CORES
    nc = tc.nc
    NP = B // 256  # pairs

    const_pool = ctx.enter_context(tc.tile_pool(name="const", bufs=1))
    io_pool = ctx.enter_context(tc.tile_pool(name="io", bufs=2))
    dig_pool = ctx.enter_context(tc.tile_pool(name="dig", bufs=2))
    kt_pool = ctx.enter_context(tc.tile_pool(name="kt", bufs=2))
    oh_pool = ctx.enter_context(tc.tile_pool(name="oh", bufs=2))
    scr_pool = ctx.enter_context(tc.tile_pool(name="scr", bufs=2))
    tp_psum = ctx.enter_context(tc.tile_pool(name="tp", bufs=2, space="PSUM"))
    mm_psum = ctx.enter_context(tc.tile_pool(name="mm", bufs=1, space="PSUM"))
    stage_pool = ctx.enter_context(tc.tile_pool(name="stage", bufs=2))

    idh = const_pool.tile([128, 128], FP16, tag="idh")
    nc.sync.dma_start(idh[:], identh_d.ap())
    nab = len(ACT_FULL) + len(ACT_HALF) + 1
    ab = const_pool.tile([128, nab], F32, tag="ab")
    nc.sync.dma_start(ab[:], actbias_d.ap())
    act_bias_col = {}
    j = 0
    for c in list(ACT_FULL) + list(ACT_HALF):
        act_bias_col[c] = j
        j += 1
    one_col = nab - 1

    ps_tiles = []
    for i in range(2):
        ps = mm_psum.tile([128, 1120], F32, tag=f"ps{i}")
        ps_tiles.append(ps)

    pend = None
    for p in range(NP):
        # ---- load 256 rows as [128, 400]: cols 0:200 tile0, 200:400 tile1
        xa = io_pool.tile([128, 2 * S], I32, tag="xa")
        src = bass.AP(x, p * 256 * S, [[S, 128], [128 * S, 2], [1, S]])
        in_eng = (nc.sync, nc.scalar)[p % 2]
        in_eng.dma_start(xa[:], src)

        # ---- digits row-major: xh = fp16(x); h = fp16(x >> 5)
        xh = dig_pool.tile([128, 2 * S], FP16, tag="xh")
        nc.vector.tensor_scalar(xh[:], xa[:], 1.0, None, AluOp.mult)
        h32 = dig_pool.tile([128, 2 * S], I32, tag="h32")
        nc.vector.tensor_scalar(h32[:], xa[:], 5, None,
                                AluOp.logical_shift_right)
        hf = dig_pool.tile([128, 2 * S], FP16, tag="hf")
        nc.gpsimd.tensor_scalar(hf[:], h32[:], 1.0, None, AluOp.mult)

        # ---- PE transposes to k-major: tp = [hT(512) | xT(512)] fp16
        tp = tp_psum.tile([128, 1024], FP16, tag="tp")
        for half in range(2):
            o = 256 * half
            c0 = S * half
            nc.tensor.transpose(tp[:, o:o + 128], hf[:, c0:c0 + 128], idh[:])
            nc.tensor.transpose(tp[0:KB, o + 128:o + 256],
                                hf[:, c0 + 128:c0 + S], idh[:])
            nc.tensor.transpose(tp[:, 512 + o:512 + o + 128],
                                xh[:, c0:c0 + 128], idh[:])
            nc.tensor.transpose(tp[0:KB, 512 + o + 128:512 + o + 256],
                                xh[:, c0 + 128:c0 + S], idh[:])

        # ---- kt = [lo(512) | hT(512) | xT(512)]
        kt = kt_pool.tile([128, 1536], FP16, tag="kt")
        nc.scalar.copy(kt[:, 512:1536], tp[:])
        nc.vector.scalar_tensor_tensor(kt[:, 0:512], kt[:, 512:1024], -32.0,
                                       kt[:, 1024:1536], AluOp.mult, AluOp.add)

        stage = stage_pool.tile([128, 2240], FP16, tag="stage")

        # ---- flush previous pair's matmuls behind this pair's PE transposes
        if pend is not None:
            _emit_pair_mms(nc, ps_tiles, *pend)

        # ---- one-hot generation (shared l/h compares)
        oh = oh_pool.tile([128, NH * 1024], FP16, tag="oh")
        dig_full = kt[:, 0:1024]
        dig_half = kt[:, 512:1024]
        for c in DVE_FULL:
            nc.vector.tensor_scalar(oh[:, 1024 * c:1024 * (c + 1)],
                                    dig_full, float(c), None, AluOp.is_equal)
        for c in POOL_FULL:
            nc.gpsimd.tensor_scalar(oh[:, 1024 * c:1024 * (c + 1)],
                                    dig_full, float(c), None, AluOp.is_equal)
        for c in ACT_FULL:
            t1 = scr_pool.tile([128, 1024], FP16, tag="t1")
            nc.scalar.activation(t1[:], dig_full, ActFn.Abs,
                                 bias=ab[:, act_bias_col[c]:act_bias_col[c] + 1])
            nc.scalar.activation(oh[:, 1024 * c:1024 * (c + 1)], t1[:],
                                 ActFn.Relu, bias=ab[:, one_col:one_col + 1],
                                 scale=-1.0)
        for c in DVE_HALF:
            nc.vector.tensor_scalar(oh[:, 1024 * c + 512:1024 * (c + 1)],
                                    dig_half, float(c), None, AluOp.is_equal)
        for c in POOL_HALF:
            nc.gpsimd.tensor_scalar(oh[:, 1024 * c + 512:1024 * (c + 1)],
                                    dig_half, float(c), None, AluOp.is_equal)
        for c in ACT_HALF:
            t1 = scr_pool.tile([128, 512], FP16, tag="t1h")
            nc.scalar.activation(t1[:], dig_half, ActFn.Abs,
                                 bias=ab[:, act_bias_col[c]:act_bias_col[c] + 1])
            nc.scalar.activation(oh[:, 1024 * c + 512:1024 * (c + 1)], t1[:],
                                 ActFn.Relu, bias=ab[:, one_col:one_col + 1],
                                 scale=-1.0)

        oh3 = oh[:].rearrange("p (b c) -> p c b", b=NH)
        pend = (stage, oh3, p, y)
    _emit_pair_mms(nc, ps_tiles, *pend)


def _build_program():
    B = FULL_B // N_CORES
    nc = bacc.Bacc("TRN2", target_bir_lowering=False, debug=False,
                   num_devices=N_CORES)
    x = nc.dram_tensor("x", [B, S], I32, kind="ExternalInput")
    identh = nc.dram_tensor("identh", [128, 128], FP16, kind="ExternalInput")
    nab = len(ACT_FULL) + len(ACT_HALF) + 1
    actbias = nc.dram_tensor("actbias", [128, nab], F32, kind="ExternalInput")
    y = nc.dram_tensor("y", [B, V], FP16, kind="ExternalOutput")
    with tile.TileContext(nc) as tc:
        with ExitStack() as ctx:
            _kernel_body(ctx, tc, y, x, identh, actbias)
    nc.compile()
    return nc


_program_cache = {}


def _get_program():
    if "nc" not in _program_cache:
        _program_cache["nc"] = _build_program()
    return _program_cache["nc"]


def kernel(**inputs) -> np.ndarray:
    B = FULL_B // N_CORES
    x_full = np.ascontiguousarray(np.asarray(inputs["inputs"], dtype=np.int32))
    assert x_full.shape == (FULL_B, S), x_full.shape

    nc = _get_program()
    consts = _host_consts()
    in_maps = []
    for c in range(N_CORES):
        m = {"x": np.ascontiguousarray(x_full[c * B:(c + 1) * B])}
        m.update(consts)
        in_maps.append(m)

    res = run_bass_kernel_spmd(nc, in_maps, core_ids=list(range(N_CORES)))
    ys = [np.asarray(res.results[c]["y"]) for c in range(N_CORES)]
    full = np.concatenate(ys, axis=0).astype(np.float32)
    # device bin order is (l, h); v = 32*h + l -> permute to v order
    full = full.reshape(FULL_B, NL, NH).transpose(0, 2, 1).reshape(FULL_B, V)
    return np.ascontiguousarray(full[:, 1:1100])


# revision 9
# speedup vs baseline: 1.2434x; 1.0082x over previous
"""Trainium2 Bass kernel: per-row bincount (BagOfWords) over 8 NeuronCores.

Problem: inputs int32 [16384, 200], values in [0, 1100); output f32
[16384, 1099] = per-row histogram over token ids 1..1099 (bin 0 dropped).

Strategy (pure data parallel): shard the batch over 8 cores (2048 rows
each). Per core, factorize each token id v = 32*h + l (h in [0,35),
l in [0,32)) and compute the per-row histogram as a tiny per-row matmul
on the PE systolic array:

    psum[l, h] = sum_j onehot_l(l_j)[l] * onehot_h(h_j)[h]

with the contraction over token slots on the partition dim (k = 128 + 72).
Digits h = x>>5 (int shift + cast) are extracted row-major and PE-transposed
to k-major alongside x (fp16); l = x - 32h is derived k-major. One-hot
matrices are generated in fp16 with per-bin compares split across the
Vector (DVE 4x mode), GPSIMD, and Activation engines; since l-bins (0..31)
and h-bins (0..34) share scalar values, one compare op covers both digit
tensors ([lo | hT] adjacent in SBUF) for bins 0..31. Matmul emission is
software-pipelined one 256-row pair behind generation. Per-row [32, 35]
results are packed 4-across-partitions (PE col-groups) densely per PSUM
tile, evicted in bulk on the Scalar engine as fp16, and DMA'd to a
[2048, 1120] output in (l-major, h) bin order; the host permutes to
v = 32h + l, drops bins 0 and 1100+, and concatenates shards. All
arithmetic is exact (integer-valued fp16/f32).
"""

import numpy as np
import ml_dtypes
from contextlib import ExitStack

import concourse.bass as bass
import concourse.tile as tile
from concourse import bacc, mybir
from concourse.bass_utils import run_bass_kernel_spmd

FP16 = mybir.dt.float16
F32 = mybir.dt.float32
I32 = mybir.dt.int32
AluOp = mybir.AluOpType
ActFn = mybir.ActivationFunctionType

N_CORES = 8
FULL_B = 16384
S = 200
NL, NH = 32, 35          # v = 32*h + l; psum out = [NL partitions, NH free]
V = NL * NH              # 1120 device bins; host drops 0 and 1100..1119
KA, KB = 128, 72

# engine split for the 35 one-hot compare ops per 256-row pair.
# full bins (l and h share the compare, [128, 1024]): c = 0..31
# half bins (h only, [128, 512]): c = 32, 33, 34
DVE_FULL = tuple(range(0, 25))
POOL_FULL = tuple(range(25, 30))
ACT_FULL = tuple(range(30, 32))
DVE_HALF = (32, 33)
ACT_HALF = (34,)
POOL_HALF = ()


def _host_consts():
    identh = np.eye(128, dtype=np.float16)
    # activation bias table: col j = -(ACT bin value), last col = +1.0
    act_bins = [float(c) for c in ACT_FULL] + [float(c) for c in ACT_HALF]
    ab = np.zeros((128, len(act_bins) + 1), dtype=np.float32)
    for j, c in enumerate(act_bins):
        ab[:, j] = -c
    ab[:, -1] = 1.0
    return {"identh": np.ascontiguousarray(identh),
            "actbias": np.ascontiguousarray(ab)}


def _emit_pair_mms(nc, ps_tiles, stage, oh3, pair, y):
    """Matmuls + eviction for one 256-row pair; oh3 = [128, 1024 cols, 35 bins]."""
    for g in range(4):
        ps = ps_tiles[g // 2]              # tile T holds half-pair rows 128T+
        goff = 560 * (g % 2)
        for r in range(64):
            rr = g * 64 + r
            s = r % 4
            q = (r // 4) % 8
            b2 = r // 32
            half = rr // 128
            rloc = rr % 128
            ca = 256 * half + rloc        # chunk A col (k = 0..127)
            cb = ca + 128                  # chunk B col (k = 128..199)
            out_ap = ps[32 * s:32 * s + NL,
                        goff + 280 * b2 + NH * q:goff + 280 * b2 + NH * q + NH]
            nc.tensor.matmul(out_ap,
                             oh3[:, ca, 0:NL],
                             oh3[:, 512 + ca, 0:NH],
                             start=True, stop=False,
                             tile_position=(0, 32 * s))
            nc.tensor.matmul(out_ap,
                             oh3[0:KB, cb, 0:NL],
                             oh3[0:KB, 512 + cb, 0:NH],
                             start=False, stop=True,
                             tile_position=(0, 32 * s))
        if g % 2 == 1:  # half-pair done: evict psum tile g//2
            t = g // 2
            nc.scalar.copy(stage[:, 1120 * t:1120 * (t + 1)],
                           ps_tiles[t][:])

    # 8 output DMAs: one per (s, psum-tile T); dst row = 128T+64G+32b2+4q+s
    # = 128T + s + 4*i with i = 16G+8b2+q matching stage col 35*i+h.
    E = pair
    for s in range(4):
        for t in range(2):
            src = stage[32 * s:32 * s + NL,
                        1120 * t:1120 * (t + 1)].rearrange(
                "p (i h) -> p i h", h=NH)
            dst = bass.AP(y, (256 * E + 128 * t + s) * V,
                          [[NH, NL], [4 * V, 32], [1, NH]])
            nc.sync.dma_start(dst, src)


def _kernel_body(ctx, tc, y, x, identh_d, actbias_d):
    B = FULL_B // N_CORES
    nc = tc.nc
    NP = B // 256  # pairs

    const_pool = ctx.enter_context(tc.tile_pool(name="const", bufs=1))
    io_pool = ctx.enter_context(tc.tile_pool(name="io", bufs=2))
    dig_pool = ctx.enter_context(tc.tile_pool(name="dig", bufs=2))
    kt_pool = ctx.enter_context(tc.tile_pool(name="kt", bufs=2))
    oh_pool = ctx.enter_context(tc.tile_pool(name="oh", bufs=2))
    scr_pool = ctx.enter_context(tc.tile_pool(name="scr", bufs=2))
    tp_psum = ctx.enter_context(tc.tile_pool(name="tp", bufs=2, space="PSUM"))
    mm_psum = ctx.enter_context(tc.tile_pool(name="mm", bufs=1, space="PSUM"))
    stage_pool = ctx.enter_context(tc.tile_pool(name="stage", bufs=2))

    idh = const_pool.tile([128, 128], FP16, tag="idh")
    nc.sync.dma_start(idh[:], identh_d.ap())
    nab = len(ACT_FULL) + len(ACT_HALF) + 1
    ab = const_pool.tile([128, nab], F32, tag="ab")
    nc.sync.dma_start(ab[:], actbias_d.ap())
    act_bias_col = {}
    j = 0
    for c in list(ACT_FULL) + list(ACT_HALF):
        act_bias_col[c] = j
        j += 1
    one_col = nab - 1

    ps_tiles = []
    for i in range(2):
        ps = mm_psum.tile([128, 1120], F32, tag=f"ps{i}")
        ps_tiles.append(ps)

    pend = None
    for p in range(NP):
        # ---- load 256 rows as [128, 400]: cols 0:200 tile0, 200:400 tile1
        xa = io_pool.tile([128, 2 * S], I32, tag="xa")
        src = bass.AP(x, p * 256 * S, [[S, 128], [128 * S, 2], [1, S]])
        nc.sync.dma_start(xa[:], src)

        # ---- digits row-major: xh = fp16(x); h = fp16(x >> 5)
        xh = dig_pool.tile([128, 2 * S], FP16, tag="xh")
        nc.vector.tensor_scalar(xh[:], xa[:], 1.0, None, AluOp.mult)
        h32 = dig_pool.tile([128, 2 * S], I32, tag="h32")
        nc.vector.tensor_scalar(h32[:], xa[:], 5, None,
                                AluOp.logical_shift_right)
        hf = dig_pool.tile([128, 2 * S], FP16, tag="hf")
        nc.gpsimd.tensor_scalar(hf[:], h32[:], 1.0, None, AluOp.mult)

        # ---- PE transposes to k-major: tp = [hT(512) | xT(512)] fp16
        tp = tp_psum.tile([128, 1024], FP16, tag="tp")
        for half in range(2):
            o = 256 * half
            c0 = S * half
            nc.tensor.transpose(tp[:, o:o + 128], hf[:, c0:c0 + 128], idh[:])
            nc.tensor.transpose(tp[0:KB, o + 128:o + 256],
                                hf[:, c0 + 128:c0 + S], idh[:])
            nc.tensor.transpose(tp[:, 512 + o:512 + o + 128],
                                xh[:, c0:c0 + 128], idh[:])
            nc.tensor.transpose(tp[0:KB, 512 + o + 128:512 + o + 256],
                                xh[:, c0 + 128:c0 + S], idh[:])

        # ---- kt = [lo(512) | hT(512) | xT(512)]
        kt = kt_pool.tile([128, 1536], FP16, tag="kt")
        nc.scalar.copy(kt[:, 512:1536], tp[:])
        nc.vector.scalar_tensor_tensor(kt[:, 0:512], kt[:, 512:1024], -32.0,
                                       kt[:, 1024:1536], AluOp.mult, AluOp.add)

        stage = stage_pool.tile([128, 2240], FP16, tag="stage")

        # ---- one-hot generation (shared l/h compares)
        oh = oh_pool.tile([128, NH * 1024], FP16, tag="oh")
        dig_full = kt[:, 0:1024]
        dig_half = kt[:, 512:1024]
        for c in DVE_FULL:
            nc.vector.tensor_scalar(oh[:, 1024 * c:1024 * (c + 1)],
                                    dig_full, float(c), None, AluOp.is_equal)
        for c in POOL_FULL:
            nc.gpsimd.tensor_scalar(oh[:, 1024 * c:1024 * (c + 1)],
                                    dig_full, float(c), None, AluOp.is_equal)
        for c in ACT_FULL:
            t1 = scr_pool.tile([128, 1024], FP16, tag="t1")
            nc.scalar.activation(t1[:], dig_full, ActFn.Abs,
                                 bias=ab[:, act_bias_col[c]:act_bias_col[c] + 1])
            nc.scalar.activation(oh[:, 1024 * c:1024 * (c + 1)], t1[:],
                                 ActFn.Relu, bias=ab[:, one_col:one_col + 1],
                                 scale=-1.0)
        for c in DVE_HALF:
            nc.vector.tensor_scalar(oh[:, 1024 * c + 512:1024 * (c + 1)],
                                    dig_half, float(c), None, AluOp.is_equal)
        for c in POOL_HALF:
            nc.gpsimd.tensor_scalar(oh[:, 1024 * c + 512:1024 * (c + 1)],
                                    dig_half, float(c), None, AluOp.is_equal)
        for c in ACT_HALF:
            t1 = scr_pool.tile([128, 512], FP16, tag="t1h")
            nc.scalar.activation(t1[:], dig_half, ActFn.Abs,
                                 bias=ab[:, act_bias_col[c]:act_bias_col[c] + 1])
            nc.scalar.activation(oh[:, 1024 * c + 512:1024 * (c + 1)], t1[:],
                                 ActFn.Relu, bias=ab[:, one_col:one_col + 1],
                                 scale=-1.0)

        # ---- flush previous pair's matmuls (its compares are already done;
        # this pair's compares were queued first so its MMs aren't gated on
        # the eviction copies sitting ahead of them in the Act queue)
        if pend is not None:
            _emit_pair_mms(nc, ps_tiles, *pend)

        oh3 = oh[:].rearrange("p (b c) -> p c b", b=NH)
        pend = (stage, oh3, p, y)
    _emit_pair_mms(nc, ps_tiles, *pend)


def _build_program():
    B = FULL_B // N_CORES
    nc = bacc.Bacc("TRN2", target_bir_lowering=False, debug=False,
                   num_devices=N_CORES)
    x = nc.dram_tensor("x", [B, S], I32, kind="ExternalInput")
    identh = nc.dram_tensor("identh", [128, 128], FP16, kind="ExternalInput")
    nab = len(ACT_FULL) + len(ACT_HALF) + 1
    actbias = nc.dram_tensor("actbias", [128, nab], F32, kind="ExternalInput")
    y = nc.dram_tensor("y", [B, V], FP16, kind="ExternalOutput")
    with tile.TileContext(nc) as tc:
        with ExitStack() as ctx:
            _kernel_body(ctx, tc, y, x, identh, actbias)
    nc.compile()
    return nc


_program_cache = {}


def _get_program():
    if "nc" not in _program_cache:
        _program_cache["nc"] = _build_program()
    return _program_cache["nc"]


def kernel(**inputs) -> np.ndarray:
    B = FULL_B // N_CORES
    x_full = np.ascontiguousarray(np.asarray(inputs["inputs"], dtype=np.int32))
    assert x_full.shape == (FULL_B, S), x_full.shape

    nc = _get_program()
    consts = _host_consts()
    in_maps = []
    for c in range(N_CORES):
        m = {"x": np.ascontiguousarray(x_full[c * B:(c + 1) * B])}
        m.update(consts)
        in_maps.append(m)

    res = run_bass_kernel_spmd(nc, in_maps, core_ids=list(range(N_CORES)))
    ys = [np.asarray(res.results[c]["y"]) for c in range(N_CORES)]
    full = np.concatenate(ys, axis=0).astype(np.float32)
    # device bin order is (l, h); v = 32*h + l -> permute to v order
    full = full.reshape(FULL_B, NL, NH).transpose(0, 2, 1).reshape(FULL_B, V)
    return np.ascontiguousarray(full[:, 1:1100])


# revision 12
# speedup vs baseline: 1.2587x; 1.0123x over previous
"""Trainium2 Bass kernel: per-row bincount (BagOfWords) over 8 NeuronCores.

Problem: inputs int32 [16384, 200], values in [0, 1100); output f32
[16384, 1099] = per-row histogram over token ids 1..1099 (bin 0 dropped).

Strategy (pure data parallel): shard the batch over 8 cores (2048 rows
each). Per core, factorize each token id v = 32*h + l (h in [0,35),
l in [0,32)) and compute the per-row histogram as a tiny per-row matmul
on the PE systolic array:

    psum[l, h] = sum_j onehot_l(l_j)[l] * onehot_h(h_j)[h]

with the contraction over token slots on the partition dim (k = 128 + 72).
Digits h = x>>5 (int shift + cast) are extracted row-major and PE-transposed
to k-major alongside x (fp16); l = x - 32h is derived k-major. One-hot
matrices are generated in fp16 with per-bin compares split across the
Vector (DVE 4x mode), GPSIMD, and Activation engines; since l-bins (0..31)
and h-bins (0..34) share scalar values, one compare op covers both digit
tensors ([lo | hT] adjacent in SBUF) for bins 0..31. Matmul emission is
software-pipelined one 256-row pair behind generation. Per-row [32, 35]
results are packed 4-across-partitions (PE col-groups) densely per PSUM
tile, evicted in bulk on the Scalar engine as fp16, and DMA'd to a
[2048, 1120] output in (l-major, h) bin order; the host permutes to
v = 32h + l, drops bins 0 and 1100+, and concatenates shards. All
arithmetic is exact (integer-valued fp16/f32).
"""

import numpy as np
import ml_dtypes
from contextlib import ExitStack

import concourse.bass as bass
import concourse.tile as tile
from concourse import bacc, mybir
from concourse.bass_utils import run_bass_kernel_spmd

FP16 = mybir.dt.float16
F32 = mybir.dt.float32
I32 = mybir.dt.int32
AluOp = mybir.AluOpType
ActFn = mybir.ActivationFunctionType

N_CORES = 8
FULL_B = 16384
S = 200
NL, NH = 32, 35          # v = 32*h + l; psum out = [NL partitions, NH free]
V = NL * NH              # 1120 device bins; host drops 0 and 1100..1119
KA, KB = 128, 72

# engine split for the 35 one-hot compare ops per 256-row pair.
# full bins (l and h share the compare, [128, 1024]): c = 0..31
# half bins (h only, [128, 512]): c = 32, 33, 34
DVE_FULL = tuple(range(0, 25))
POOL_FULL = tuple(range(25, 30))
ACT_FULL = tuple(range(30, 32))
DVE_HALF = (32, 33)
ACT_HALF = (34,)
POOL_HALF = ()


def _host_consts():
    identh = np.eye(128, dtype=np.float16)
    # activation bias table: col j = -(ACT bin value), last col = +1.0
    act_bins = [float(c) for c in ACT_FULL] + [float(c) for c in ACT_HALF]
    ab = np.zeros((128, len(act_bins) + 1), dtype=np.float32)
    for j, c in enumerate(act_bins):
        ab[:, j] = -c
    ab[:, -1] = 1.0
    return {"identh": np.ascontiguousarray(identh),
            "actbias": np.ascontiguousarray(ab)}


def _emit_pair_mms(nc, ps_tiles, stage, oh3, pair, y):
    """Matmuls + eviction for one 256-row pair; oh3 = [128, 1024 cols, 35 bins]."""
    for g in range(4):
        ps = ps_tiles[g // 2]              # tile T holds half-pair rows 128T+
        goff = 560 * (g % 2)
        for r in range(64):
            rr = g * 64 + r
            s = r % 4
            q = (r // 4) % 8
            b2 = r // 32
            half = rr // 128
            rloc = rr % 128
            ca = 256 * half + rloc        # chunk A col (k = 0..127)
            cb = ca + 128                  # chunk B col (k = 128..199)
            out_ap = ps[32 * s:32 * s + NL,
                        goff + 280 * b2 + NH * q:goff + 280 * b2 + NH * q + NH]
            nc.tensor.matmul(out_ap,
                             oh3[:, ca, 0:NL],
                             oh3[:, 512 + ca, 0:NH],
                             start=True, stop=False,
                             tile_position=(0, 32 * s))
            nc.tensor.matmul(out_ap,
                             oh3[0:KB, cb, 0:NL],
                             oh3[0:KB, 512 + cb, 0:NH],
                             start=False, stop=True,
                             tile_position=(0, 32 * s))
        if g % 2 == 1:  # half-pair done: evict psum tile g//2
            t = g // 2
            nc.scalar.copy(stage[:, 1120 * t:1120 * (t + 1)],
                           ps_tiles[t][:])

    # 4 output DMAs, one per s: dst row = 256E + s + 4*i' where
    # i' = 32T + 16G + 8b2 + q matches stage col 35*i' + h exactly.
    E = pair
    for s in range(4):
        src = stage[32 * s:32 * s + NL, :].rearrange("p (i h) -> p i h", h=NH)
        dst = bass.AP(y, (256 * E + s) * V, [[NH, NL], [4 * V, 64], [1, NH]])
        nc.sync.dma_start(dst, src)


def _kernel_body(ctx, tc, y, x, identh_d, actbias_d):
    B = FULL_B // N_CORES
    nc = tc.nc
    NP = B // 256  # pairs

    const_pool = ctx.enter_context(tc.tile_pool(name="const", bufs=1))
    io_pool = ctx.enter_context(tc.tile_pool(name="io", bufs=3))
    dig_pool = ctx.enter_context(tc.tile_pool(name="dig", bufs=2))
    kt_pool = ctx.enter_context(tc.tile_pool(name="kt", bufs=2))
    oh_pool = ctx.enter_context(tc.tile_pool(name="oh", bufs=2))
    scr_pool = ctx.enter_context(tc.tile_pool(name="scr", bufs=2))
    tp_psum = ctx.enter_context(tc.tile_pool(name="tp", bufs=2, space="PSUM"))
    mm_psum = ctx.enter_context(tc.tile_pool(name="mm", bufs=1, space="PSUM"))
    stage_pool = ctx.enter_context(tc.tile_pool(name="stage", bufs=2))

    idh = const_pool.tile([128, 128], FP16, tag="idh")
    nc.sync.dma_start(idh[:], identh_d.ap())
    nab = len(ACT_FULL) + len(ACT_HALF) + 1
    ab = const_pool.tile([128, nab], F32, tag="ab")
    nc.sync.dma_start(ab[:], actbias_d.ap())
    act_bias_col = {}
    j = 0
    for c in list(ACT_FULL) + list(ACT_HALF):
        act_bias_col[c] = j
        j += 1
    one_col = nab - 1

    ps_tiles = []
    for i in range(2):
        ps = mm_psum.tile([128, 1120], F32, tag=f"ps{i}")
        ps_tiles.append(ps)

    def load_pair(p):
        # load 256 rows as [128, 400]: cols 0:200 tile0, 200:400 tile1
        xa = io_pool.tile([128, 2 * S], I32, tag="xa")
        src = bass.AP(x, p * 256 * S, [[S, 128], [128 * S, 2], [1, S]])
        nc.sync.dma_start(xa[:], src)
        return xa

    pend = None
    xa_next = load_pair(0)
    for p in range(NP):
        xa = xa_next
        if p + 1 < NP:
            # prefetch next pair's rows ahead of this flush's output DMAs so
            # the load isn't queued behind sems waiting on eviction copies
            xa_next = load_pair(p + 1)

        # ---- digits row-major: xh = fp16(x); h = fp16(x >> 5)
        xh = dig_pool.tile([128, 2 * S], FP16, tag="xh")
        nc.vector.tensor_scalar(xh[:], xa[:], 1.0, None, AluOp.mult)
        h32 = dig_pool.tile([128, 2 * S], I32, tag="h32")
        nc.vector.tensor_scalar(h32[:], xa[:], 5, None,
                                AluOp.logical_shift_right)
        hf = dig_pool.tile([128, 2 * S], FP16, tag="hf")
        nc.gpsimd.tensor_scalar(hf[:], h32[:], 1.0, None, AluOp.mult)

        # ---- PE transposes to k-major: tp = [hT(512) | xT(512)] fp16
        tp = tp_psum.tile([128, 1024], FP16, tag="tp")
        for half in range(2):
            o = 256 * half
            c0 = S * half
            nc.tensor.transpose(tp[:, o:o + 128], hf[:, c0:c0 + 128], idh[:])
            nc.tensor.transpose(tp[0:KB, o + 128:o + 256],
                                hf[:, c0 + 128:c0 + S], idh[:])
            nc.tensor.transpose(tp[:, 512 + o:512 + o + 128],
                                xh[:, c0:c0 + 128], idh[:])
            nc.tensor.transpose(tp[0:KB, 512 + o + 128:512 + o + 256],
                                xh[:, c0 + 128:c0 + S], idh[:])

        # ---- kt = [lo(512) | hT(512) | xT(512)]
        kt = kt_pool.tile([128, 1536], FP16, tag="kt")
        nc.scalar.copy(kt[:, 512:1536], tp[:])
        nc.vector.scalar_tensor_tensor(kt[:, 0:512], kt[:, 512:1024], -32.0,
                                       kt[:, 1024:1536], AluOp.mult, AluOp.add)

        stage = stage_pool.tile([128, 2240], FP16, tag="stage")

        # ---- one-hot generation (shared l/h compares)
        oh = oh_pool.tile([128, NH * 1024], FP16, tag="oh")
        dig_full = kt[:, 0:1024]
        dig_half = kt[:, 512:1024]
        for c in DVE_FULL:
            nc.vector.tensor_scalar(oh[:, 1024 * c:1024 * (c + 1)],
                                    dig_full, float(c), None, AluOp.is_equal)
        for c in POOL_FULL:
            nc.gpsimd.tensor_scalar(oh[:, 1024 * c:1024 * (c + 1)],
                                    dig_full, float(c), None, AluOp.is_equal)
        for c in ACT_FULL:
            t1 = scr_pool.tile([128, 1024], FP16, tag="t1")
            nc.scalar.activation(t1[:], dig_full, ActFn.Abs,
                                 bias=ab[:, act_bias_col[c]:act_bias_col[c] + 1])
            nc.scalar.activation(oh[:, 1024 * c:1024 * (c + 1)], t1[:],
                                 ActFn.Relu, bias=ab[:, one_col:one_col + 1],
                                 scale=-1.0)
        for c in DVE_HALF:
            nc.vector.tensor_scalar(oh[:, 1024 * c + 512:1024 * (c + 1)],
                                    dig_half, float(c), None, AluOp.is_equal)
        for c in POOL_HALF:
            nc.gpsimd.tensor_scalar(oh[:, 1024 * c + 512:1024 * (c + 1)],
                                    dig_half, float(c), None, AluOp.is_equal)
        for c in ACT_HALF:
            t1 = scr_pool.tile([128, 512], FP16, tag="t1h")
            nc.scalar.activation(t1[:], dig_half, ActFn.Abs,
                                 bias=ab[:, act_bias_col[c]:act_bias_col[c] + 1])
            nc.scalar.activation(oh[:, 1024 * c + 512:1024 * (c + 1)], t1[:],
                                 ActFn.Relu, bias=ab[:, one_col:one_col + 1],
                                 scale=-1.0)

        # ---- flush previous pair's matmuls (its compares are already done;
        # this pair's compares were queued first so its MMs aren't gated on
        # the eviction copies sitting ahead of them in the Act queue)
        if pend is not None:
            _emit_pair_mms(nc, ps_tiles, *pend)

        oh3 = oh[:].rearrange("p (b c) -> p c b", b=NH)
        pend = (stage, oh3, p, y)
    _emit_pair_mms(nc, ps_tiles, *pend)


def _build_program():
    B = FULL_B // N_CORES
    nc = bacc.Bacc("TRN2", target_bir_lowering=False, debug=False,
                   num_devices=N_CORES)
    x = nc.dram_tensor("x", [B, S], I32, kind="ExternalInput")
    identh = nc.dram_tensor("identh", [128, 128], FP16, kind="ExternalInput")
    nab = len(ACT_FULL) + len(ACT_HALF) + 1
    actbias = nc.dram_tensor("actbias", [128, nab], F32, kind="ExternalInput")
    y = nc.dram_tensor("y", [B, V], FP16, kind="ExternalOutput")
    with tile.TileContext(nc) as tc:
        with ExitStack() as ctx:
            _kernel_body(ctx, tc, y, x, identh, actbias)
    nc.compile()
    return nc


_program_cache = {}


def _get_program():
    if "nc" not in _program_cache:
        _program_cache["nc"] = _build_program()
    return _program_cache["nc"]


def kernel(**inputs) -> np.ndarray:
    B = FULL_B // N_CORES
    x_full = np.ascontiguousarray(np.asarray(inputs["inputs"], dtype=np.int32))
    assert x_full.shape == (FULL_B, S), x_full.shape

    nc = _get_program()
    consts = _host_consts()
    in_maps = []
    for c in range(N_CORES):
        m = {"x": np.ascontiguousarray(x_full[c * B:(c + 1) * B])}
        m.update(consts)
        in_maps.append(m)

    res = run_bass_kernel_spmd(nc, in_maps, core_ids=list(range(N_CORES)))
    ys = [np.asarray(res.results[c]["y"]) for c in range(N_CORES)]
    full = np.concatenate(ys, axis=0).astype(np.float32)
    # device bin order is (l, h); v = 32*h + l -> permute to v order
    full = full.reshape(FULL_B, NL, NH).transpose(0, 2, 1).reshape(FULL_B, V)
    return np.ascontiguousarray(full[:, 1:1100])


# revision 17
# speedup vs baseline: 1.4449x; 1.1479x over previous
"""Trainium2 Bass kernel: per-row bincount (BagOfWords) over 8 NeuronCores.

Problem: inputs int32 [16384, 200], values in [0, 1100); output f32
[16384, 1099] = per-row histogram over token ids 1..1099 (bin 0 dropped).

Strategy (pure data parallel): shard the batch over 8 cores (2048 rows
each), staging tokens as int16 (values < 2048). Per core, factorize each
token id v = 32*h + l (h in [0,35), l in [0,32)) and compute the per-row
histogram as a tiny per-row matmul on the PE systolic array:

    psum[l, h] = sum_j onehot_l(l_j)[l] * onehot_h(h_j)[h]

with the contraction over token slots on the partition dim (k = 128 + 72).
Tokens are transposed to k-major with XBAR DMA transposes (16-bit path),
digits extracted k-major on the Vector engine (int shift + fused
multiply-add, all exact), and one-hot matrices generated in fp16 with
per-bin compares split across the Vector (DVE 4x mode), GPSIMD, and
Activation engines; since l-bins (0..31) and h-bins (0..34) share scalar
values, one compare op covers both digit tensors ([lo | h] adjacent in
SBUF) for bins 0..31. Matmul emission is software-pipelined one 256-row
pair behind one-hot generation, which itself runs one pair behind the
load/transpose/digit front-end. Per-row [32, 35] results are packed
4-across-partitions (PE col-groups) densely per PSUM tile, evicted in
bulk on the Scalar engine as fp16, and DMA'd to a [2048, 1120] output in
(l-major, h) bin order; the host permutes to v = 32h + l, drops bins 0
and 1100+, and concatenates shards. All arithmetic is exact
(integer-valued int16/fp16/f32).
"""

import numpy as np
from contextlib import ExitStack

import concourse.bass as bass
import concourse.tile as tile
from concourse import bacc, mybir
from concourse.bass_utils import run_bass_kernel_spmd

FP16 = mybir.dt.float16
F32 = mybir.dt.float32
I16 = mybir.dt.int16
AluOp = mybir.AluOpType
ActFn = mybir.ActivationFunctionType

N_CORES = 8
FULL_B = 16384
S = 200
NL, NH = 32, 35          # v = 32*h + l; psum out = [NL partitions, NH free]
V = NL * NH              # 1120 device bins; host drops 0 and 1100..1119
KA, KB = 128, 72

# engine split for the 35 one-hot compare ops per 256-row pair.
# full bins (l and h share the compare, [128, 1024]): c = 0..31
# half bins (h only, [128, 512]): c = 32, 33, 34
DVE_FULL = tuple(range(0, 24))
POOL_FULL = tuple(range(24, 29))
ACT_FULL = tuple(range(29, 32))
DVE_HALF = (32, 33)
POOL_HALF = (34,)
ACT_HALF = ()


def _host_consts():
    # activation bias table: col j = -(ACT bin value), last col = +1.0
    act_bins = [float(c) for c in ACT_FULL] + [float(c) for c in ACT_HALF]
    ab = np.zeros((128, len(act_bins) + 1), dtype=np.float32)
    for j, c in enumerate(act_bins):
        ab[:, j] = -c
    ab[:, -1] = 1.0
    return {"actbias": np.ascontiguousarray(ab)}


def _emit_pair_mms(nc, ps_tiles, oh3):
    """Matmuls for one 256-row pair; oh3 = [128, 1024 cols, 35 bins]."""
    for g in range(4):
        ps = ps_tiles[g // 2]              # tile T holds half-pair rows 128T+
        goff = 560 * (g % 2)
        for r in range(64):
            rr = g * 64 + r
            s = r % 4
            q = (r // 4) % 8
            b2 = r // 32
            half = rr // 128
            rloc = rr % 128
            ca = 256 * half + rloc        # chunk A col (k = 0..127)
            cb = ca + 128                  # chunk B col (k = 128..199)
            out_ap = ps[32 * s:32 * s + NL,
                        goff + 280 * b2 + NH * q:goff + 280 * b2 + NH * q + NH]
            nc.tensor.matmul(out_ap,
                             oh3[:, ca, 0:NL],
                             oh3[:, 512 + ca, 0:NH],
                             start=True, stop=False,
                             tile_position=(0, 32 * s))
            nc.tensor.matmul(out_ap,
                             oh3[0:KB, cb, 0:NL],
                             oh3[0:KB, 512 + cb, 0:NH],
                             start=False, stop=True,
                             tile_position=(0, 32 * s))


def _emit_pair_evict(nc, ps_tiles, stage, pair, y):
    """Stage copies (Act) + output DMAs (SP) for the pair just matmul'd."""
    for t in range(2):
        nc.scalar.copy(stage[:, 1120 * t:1120 * (t + 1)], ps_tiles[t][:])
    # 4 output DMAs, one per s: dst row = 256E + s + 4*i' where
    # i' = 32T + 16G + 8b2 + q matches stage col 35*i' + h exactly.
    E = pair
    for s in range(4):
        src = stage[32 * s:32 * s + NL, :].rearrange("p (i h) -> p i h", h=NH)
        dst = bass.AP(y, (256 * E + s) * V, [[NH, NL], [4 * V, 64], [1, NH]])
        nc.sync.dma_start(dst, src)


def _kernel_body(ctx, tc, y, x, actbias_d):
    B = FULL_B // N_CORES
    nc = tc.nc
    NP = B // 256  # pairs

    const_pool = ctx.enter_context(tc.tile_pool(name="const", bufs=1))
    io_pool = ctx.enter_context(tc.tile_pool(name="io", bufs=3))
    kt_pool = ctx.enter_context(tc.tile_pool(name="kt", bufs=2))
    dig_pool = ctx.enter_context(tc.tile_pool(name="dig", bufs=2))
    oh_pool = ctx.enter_context(tc.tile_pool(name="oh", bufs=2))
    scr_pool = ctx.enter_context(tc.tile_pool(name="scr", bufs=2))
    mm_psum = ctx.enter_context(tc.tile_pool(name="mm", bufs=1, space="PSUM"))
    stage_pool = ctx.enter_context(tc.tile_pool(name="stage", bufs=2))

    nab = len(ACT_FULL) + len(ACT_HALF) + 1
    ab = const_pool.tile([128, nab], F32, tag="ab")
    nc.sync.dma_start(ab[:], actbias_d.ap())
    act_bias_col = {}
    for j, c in enumerate(list(ACT_FULL) + list(ACT_HALF)):
        act_bias_col[c] = j
    one_col = nab - 1

    ps_tiles = []
    for i in range(2):
        ps = mm_psum.tile([128, 1120], F32, tag=f"ps{i}")
        ps_tiles.append(ps)

    def load_pair(p):
        # 256 rows as [128, 512]: cols 0:200 tile0, 256:456 tile1 (padded so
        # each 128-col block can go through the XBAR transpose whole)
        xa = io_pool.tile([128, 512], I16, tag="xa")
        dst = xa[:].rearrange("p (t c) -> p t c", t=2)[:, :, 0:S]
        src = bass.AP(x, p * 256 * S, [[S, 128], [128 * S, 2], [1, S]])
        nc.sync.dma_start(dst, src)
        return xa

    def transposes(xa):
        # XBAR DMA transposes: 4 chunks of [128, 128] int16 to k-major
        ktx = kt_pool.tile([128, 512], I16, tag="ktx")
        for c in range(4):
            nc.scalar.dma_start_transpose(ktx[:, 128 * c:128 * (c + 1)],
                                          xa[:, 128 * c:128 * (c + 1)])
        return ktx

    def digits(ktx):
        # k-major digits: h = x >> 5 (int16), dig = [lo | h] as fp16
        h16 = dig_pool.tile([128, 512], I16, tag="h16")
        nc.vector.tensor_scalar(h16[:], ktx[:], 5, None,
                                AluOp.logical_shift_right)
        dig = dig_pool.tile([128, 1024], FP16, tag="dig")
        nc.vector.tensor_scalar(dig[:, 512:1024], h16[:], 1.0, None,
                                AluOp.mult)
        nc.vector.scalar_tensor_tensor(dig[:, 0:512], h16[:], -32.0, ktx[:],
                                       AluOp.mult, AluOp.add)
        return dig

    def compares(dig):
        oh = oh_pool.tile([128, NH * 1024], FP16, tag="oh")
        dig_full = dig[:, 0:1024]
        dig_half = dig[:, 512:1024]
        for c in ACT_FULL:  # first on the Act queue, ahead of evictions
            t1 = scr_pool.tile([128, 1024], FP16, tag="t1")
            nc.scalar.activation(t1[:], dig_full, ActFn.Abs,
                                 bias=ab[:, act_bias_col[c]:act_bias_col[c] + 1])
            nc.scalar.activation(oh[:, 1024 * c:1024 * (c + 1)], t1[:],
                                 ActFn.Relu, bias=ab[:, one_col:one_col + 1],
                                 scale=-1.0)
        for c in ACT_HALF:
            t1 = scr_pool.tile([128, 512], FP16, tag="t1h")
            nc.scalar.activation(t1[:], dig_half, ActFn.Abs,
                                 bias=ab[:, act_bias_col[c]:act_bias_col[c] + 1])
            nc.scalar.activation(oh[:, 1024 * c + 512:1024 * (c + 1)], t1[:],
                                 ActFn.Relu, bias=ab[:, one_col:one_col + 1],
                                 scale=-1.0)
        for c in DVE_FULL:
            nc.vector.tensor_scalar(oh[:, 1024 * c:1024 * (c + 1)],
                                    dig_full, float(c), None, AluOp.is_equal)
        for c in DVE_HALF:
            nc.vector.tensor_scalar(oh[:, 1024 * c + 512:1024 * (c + 1)],
                                    dig_half, float(c), None, AluOp.is_equal)
        for c in POOL_FULL:
            nc.gpsimd.tensor_scalar(oh[:, 1024 * c:1024 * (c + 1)],
                                    dig_full, float(c), None, AluOp.is_equal)
        for c in POOL_HALF:
            nc.gpsimd.tensor_scalar(oh[:, 1024 * c + 512:1024 * (c + 1)],
                                    dig_half, float(c), None, AluOp.is_equal)
        return oh[:].rearrange("p (b c) -> p c b", b=NH)

    # ---- software pipeline: loads 2 ahead, transpose/digits 1 ahead,
    # matmuls + eviction 1 behind the compares.
    xa_bufs = {0: load_pair(0), 1: load_pair(1)}
    dig_cur = digits(transposes(xa_bufs[0]))
    pend = None
    for p in range(NP):
        ktx_n = transposes(xa_bufs[p + 1]) if p + 1 < NP else None
        if p + 2 < NP:
            xa_bufs[p + 2] = load_pair(p + 2)
        xa_bufs.pop(p, None)
        if pend is not None:
            _emit_pair_mms(nc, ps_tiles, pend[1])
        oh3 = compares(dig_cur)
        if ktx_n is not None:
            dig_cur = digits(ktx_n)
        if pend is not None:
            _emit_pair_evict(nc, ps_tiles, pend[0], pend[2], y)
        stage = stage_pool.tile([128, 2240], FP16, tag="stage")
        pend = (stage, oh3, p)
    _emit_pair_mms(nc, ps_tiles, pend[1])
    _emit_pair_evict(nc, ps_tiles, pend[0], pend[2], y)


def _build_program():
    B = FULL_B // N_CORES
    nc = bacc.Bacc("TRN2", target_bir_lowering=False, debug=False,
                   num_devices=N_CORES)
    x = nc.dram_tensor("x", [B, S], I16, kind="ExternalInput")
    nab = len(ACT_FULL) + len(ACT_HALF) + 1
    actbias = nc.dram_tensor("actbias", [128, nab], F32, kind="ExternalInput")
    y = nc.dram_tensor("y", [B, V], FP16, kind="ExternalOutput")
    with tile.TileContext(nc) as tc:
        with ExitStack() as ctx:
            _kernel_body(ctx, tc, y, x, actbias)
    nc.compile()
    return nc


_program_cache = {}


def _get_program():
    if "nc" not in _program_cache:
        _program_cache["nc"] = _build_program()
    return _program_cache["nc"]


def kernel(**inputs) -> np.ndarray:
    B = FULL_B // N_CORES
    x_full = np.asarray(inputs["inputs"])
    assert x_full.shape == (FULL_B, S), x_full.shape
    x16 = np.ascontiguousarray(x_full.astype(np.int16))

    nc = _get_program()
    consts = _host_consts()
    in_maps = []
    for c in range(N_CORES):
        m = {"x": np.ascontiguousarray(x16[c * B:(c + 1) * B])}
        m.update(consts)
        in_maps.append(m)

    res = run_bass_kernel_spmd(nc, in_maps, core_ids=list(range(N_CORES)))
    ys = [np.asarray(res.results[c]["y"]) for c in range(N_CORES)]
    full = np.concatenate(ys, axis=0).astype(np.float32)
    # device bin order is (l, h); v = 32*h + l -> permute to v order
    full = full.reshape(FULL_B, NL, NH).transpose(0, 2, 1).reshape(FULL_B, V)
    return np.ascontiguousarray(full[:, 1:1100])


# revision 20
# speedup vs baseline: 1.4633x; 1.0128x over previous
"""Trainium2 Bass kernel: per-row bincount (BagOfWords) over 8 NeuronCores.

Problem: inputs int32 [16384, 200], values in [0, 1100); output f32
[16384, 1099] = per-row histogram over token ids 1..1099 (bin 0 dropped).

Strategy (pure data parallel): shard the batch over 8 cores (2048 rows
each), staging tokens as int16 (values < 2048). Per core, factorize each
token id v = 32*h + l (h in [0,35), l in [0,32)) and compute the per-row
histogram as a tiny per-row matmul on the PE systolic array:

    psum[l, h] = sum_j onehot_l(l_j)[l] * onehot_h(h_j)[h]

with the contraction over token slots on the partition dim (k = 128 + 72).
Tokens are transposed to k-major with XBAR DMA transposes (16-bit path),
digits extracted k-major on the Vector engine (int shift + fused
multiply-add, all exact), and one-hot matrices generated in fp16 with
per-bin compares split across the Vector (DVE 4x mode), GPSIMD, and
Activation engines; since l-bins (0..31) and h-bins (0..34) share scalar
values, one compare op covers both digit tensors ([lo | h] adjacent in
SBUF) for bins 0..31. Matmul emission is software-pipelined one 256-row
pair behind one-hot generation, which itself runs one pair behind the
load/transpose/digit front-end. Per-row [32, 35] results are packed
4-across-partitions (PE col-groups) densely per PSUM tile, evicted in
bulk on the Scalar engine as fp16, and DMA'd to a [2048, 1120] output in
(l-major, h) bin order; the host permutes to v = 32h + l, drops bins 0
and 1100+, and concatenates shards. All arithmetic is exact
(integer-valued int16/fp16/f32).
"""

import numpy as np
from contextlib import ExitStack

import concourse.bass as bass
import concourse.tile as tile
from concourse import bacc, mybir
from concourse.bass_utils import run_bass_kernel_spmd

FP16 = mybir.dt.float16
F32 = mybir.dt.float32
I16 = mybir.dt.int16
AluOp = mybir.AluOpType
ActFn = mybir.ActivationFunctionType

N_CORES = 8
FULL_B = 16384
S = 200
NL, NH = 32, 35          # v = 32*h + l; psum out = [NL partitions, NH free]
V = NL * NH              # 1120 device bins; host drops 0 and 1100..1119
KA, KB = 128, 72

# engine split for the 35 one-hot compare ops per 256-row pair.
# full bins (l and h share the compare, [128, 1024]): c = 0..31
# half bins (h only, [128, 512]): c = 32, 33, 34
DVE_FULL = tuple(range(0, 24))
POOL_FULL = tuple(range(24, 29))
ACT_FULL = tuple(range(29, 32))
DVE_HALF = (32, 33)
POOL_HALF = (34,)
ACT_HALF = ()


def _host_consts():
    # activation bias table: col j = -(ACT bin value), last col = +1.0
    act_bins = [float(c) for c in ACT_FULL] + [float(c) for c in ACT_HALF]
    ab = np.zeros((128, len(act_bins) + 1), dtype=np.float32)
    for j, c in enumerate(act_bins):
        ab[:, j] = -c
    ab[:, -1] = 1.0
    return {"actbias": np.ascontiguousarray(ab)}


def _emit_pair_mms(nc, ps_tiles, oh3):
    """Matmuls for one 256-row pair; oh3 = [128, 1024 cols, 35 bins]."""
    for g in range(4):
        ps = ps_tiles[g // 2]              # tile T holds half-pair rows 128T+
        goff = 560 * (g % 2)
        for r in range(64):
            rr = g * 64 + r
            s = r % 4
            q = (r // 4) % 8
            b2 = r // 32
            half = rr // 128
            rloc = rr % 128
            ca = 256 * half + rloc        # chunk A col (k = 0..127)
            cb = ca + 128                  # chunk B col (k = 128..199)
            out_ap = ps[32 * s:32 * s + NL,
                        goff + 280 * b2 + NH * q:goff + 280 * b2 + NH * q + NH]
            nc.tensor.matmul(out_ap,
                             oh3[:, ca, 0:NL],
                             oh3[:, 512 + ca, 0:NH],
                             start=True, stop=False,
                             tile_position=(0, 32 * s))
            nc.tensor.matmul(out_ap,
                             oh3[0:KB, cb, 0:NL],
                             oh3[0:KB, 512 + cb, 0:NH],
                             start=False, stop=True,
                             tile_position=(0, 32 * s))


def _emit_pair_evict(nc, ps_tiles, stage, pair, y, split=False):
    """Stage copies (Act) + output DMAs (SP) for the pair just matmul'd."""
    for t in range(2):
        nc.scalar.copy(stage[:, 1120 * t:1120 * (t + 1)], ps_tiles[t][:])
    E = pair
    if split:
        # tail: per-(s, T) DMAs so T=0 rows ship before T=1 is staged
        for t in range(2):
            for s in range(4):
                src = stage[32 * s:32 * s + NL,
                            1120 * t:1120 * (t + 1)].rearrange(
                    "p (i h) -> p i h", h=NH)
                dst = bass.AP(y, (256 * E + 128 * t + s) * V,
                              [[NH, NL], [4 * V, 32], [1, NH]])
                nc.sync.dma_start(dst, src)
        return
    # 4 output DMAs, one per s: dst row = 256E + s + 4*i' where
    # i' = 32T + 16G + 8b2 + q matches stage col 35*i' + h exactly.
    for s in range(4):
        src = stage[32 * s:32 * s + NL, :].rearrange("p (i h) -> p i h", h=NH)
        dst = bass.AP(y, (256 * E + s) * V, [[NH, NL], [4 * V, 64], [1, NH]])
        nc.sync.dma_start(dst, src)


def _kernel_body(ctx, tc, y, x, actbias_d):
    B = FULL_B // N_CORES
    nc = tc.nc
    NP = B // 256  # pairs

    const_pool = ctx.enter_context(tc.tile_pool(name="const", bufs=1))
    io_pool = ctx.enter_context(tc.tile_pool(name="io", bufs=3))
    kt_pool = ctx.enter_context(tc.tile_pool(name="kt", bufs=2))
    dig_pool = ctx.enter_context(tc.tile_pool(name="dig", bufs=2))
    oh_pool = ctx.enter_context(tc.tile_pool(name="oh", bufs=2))
    scr_pool = ctx.enter_context(tc.tile_pool(name="scr", bufs=2))
    mm_psum = ctx.enter_context(tc.tile_pool(name="mm", bufs=1, space="PSUM"))
    stage_pool = ctx.enter_context(tc.tile_pool(name="stage", bufs=2))

    nab = len(ACT_FULL) + len(ACT_HALF) + 1
    ab = const_pool.tile([128, nab], F32, tag="ab")
    nc.sync.dma_start(ab[:], actbias_d.ap())
    act_bias_col = {}
    for j, c in enumerate(list(ACT_FULL) + list(ACT_HALF)):
        act_bias_col[c] = j
    one_col = nab - 1

    ps_tiles = []
    for i in range(2):
        ps = mm_psum.tile([128, 1120], F32, tag=f"ps{i}")
        ps_tiles.append(ps)

    def load_pair(p):
        # 256 rows as [128, 512]: cols 0:200 tile0, 256:456 tile1 (padded so
        # each 128-col block can go through the XBAR transpose whole)
        xa = io_pool.tile([128, 512], I16, tag="xa")
        dst = xa[:].rearrange("p (t c) -> p t c", t=2)[:, :, 0:S]
        src = bass.AP(x, p * 256 * S, [[S, 128], [128 * S, 2], [1, S]])
        nc.sync.dma_start(dst, src)
        return xa

    def transposes(xa, fill=False):
        # XBAR DMA transposes: 4 chunks of [128, 128] int16 to k-major.
        # During pipeline fill spread them over two SEQs to halve latency.
        ktx = kt_pool.tile([128, 512], I16, tag="ktx")
        for c in range(4):
            eng = (nc.scalar, nc.sync)[c % 2] if fill else nc.scalar
            eng.dma_start_transpose(ktx[:, 128 * c:128 * (c + 1)],
                                    xa[:, 128 * c:128 * (c + 1)])
        return ktx

    def digits(ktx):
        # k-major digits: h = x >> 5 (int16), dig = [lo | h] as fp16
        h16 = dig_pool.tile([128, 512], I16, tag="h16")
        nc.vector.tensor_scalar(h16[:], ktx[:], 5, None,
                                AluOp.logical_shift_right)
        dig = dig_pool.tile([128, 1024], FP16, tag="dig")
        nc.vector.tensor_scalar(dig[:, 512:1024], h16[:], 1.0, None,
                                AluOp.mult)
        nc.vector.scalar_tensor_tensor(dig[:, 0:512], h16[:], -32.0, ktx[:],
                                       AluOp.mult, AluOp.add)
        return dig

    def compares(dig):
        oh = oh_pool.tile([128, NH * 1024], FP16, tag="oh")
        dig_full = dig[:, 0:1024]
        dig_half = dig[:, 512:1024]
        for c in ACT_FULL:  # first on the Act queue, ahead of evictions
            t1 = scr_pool.tile([128, 1024], FP16, tag="t1")
            nc.scalar.activation(t1[:], dig_full, ActFn.Abs,
                                 bias=ab[:, act_bias_col[c]:act_bias_col[c] + 1])
            nc.scalar.activation(oh[:, 1024 * c:1024 * (c + 1)], t1[:],
                                 ActFn.Relu, bias=ab[:, one_col:one_col + 1],
                                 scale=-1.0)
        for c in ACT_HALF:
            t1 = scr_pool.tile([128, 512], FP16, tag="t1h")
            nc.scalar.activation(t1[:], dig_half, ActFn.Abs,
                                 bias=ab[:, act_bias_col[c]:act_bias_col[c] + 1])
            nc.scalar.activation(oh[:, 1024 * c + 512:1024 * (c + 1)], t1[:],
                                 ActFn.Relu, bias=ab[:, one_col:one_col + 1],
                                 scale=-1.0)
        for c in DVE_FULL:
            nc.vector.tensor_scalar(oh[:, 1024 * c:1024 * (c + 1)],
                                    dig_full, float(c), None, AluOp.is_equal)
        for c in DVE_HALF:
            nc.vector.tensor_scalar(oh[:, 1024 * c + 512:1024 * (c + 1)],
                                    dig_half, float(c), None, AluOp.is_equal)
        for c in POOL_FULL:
            nc.gpsimd.tensor_scalar(oh[:, 1024 * c:1024 * (c + 1)],
                                    dig_full, float(c), None, AluOp.is_equal)
        for c in POOL_HALF:
            nc.gpsimd.tensor_scalar(oh[:, 1024 * c + 512:1024 * (c + 1)],
                                    dig_half, float(c), None, AluOp.is_equal)
        return oh[:].rearrange("p (b c) -> p c b", b=NH)

    # ---- software pipeline: loads 2 ahead, transpose/digits 1 ahead,
    # matmuls + eviction 1 behind the compares.
    xa_bufs = {0: load_pair(0), 1: load_pair(1)}
    dig_cur = digits(transposes(xa_bufs[0], fill=True))
    pend = None
    for p in range(NP):
        ktx_n = transposes(xa_bufs[p + 1], fill=(p == 0)) if p + 1 < NP else None
        if p + 2 < NP:
            xa_bufs[p + 2] = load_pair(p + 2)
        xa_bufs.pop(p, None)
        if pend is not None:
            _emit_pair_mms(nc, ps_tiles, pend[1])
        oh3 = compares(dig_cur)
        if ktx_n is not None:
            dig_cur = digits(ktx_n)
        if pend is not None:
            _emit_pair_evict(nc, ps_tiles, pend[0], pend[2], y)
        stage = stage_pool.tile([128, 2240], FP16, tag="stage")
        pend = (stage, oh3, p)
    _emit_pair_mms(nc, ps_tiles, pend[1])
    _emit_pair_evict(nc, ps_tiles, pend[0], pend[2], y, split=True)


def _build_program():
    B = FULL_B // N_CORES
    nc = bacc.Bacc("TRN2", target_bir_lowering=False, debug=False,
                   num_devices=N_CORES)
    x = nc.dram_tensor("x", [B, S], I16, kind="ExternalInput")
    nab = len(ACT_FULL) + len(ACT_HALF) + 1
    actbias = nc.dram_tensor("actbias", [128, nab], F32, kind="ExternalInput")
    y = nc.dram_tensor("y", [B, V], FP16, kind="ExternalOutput")
    with tile.TileContext(nc) as tc:
        with ExitStack() as ctx:
            _kernel_body(ctx, tc, y, x, actbias)
    nc.compile()
    return nc


_program_cache = {}


def _get_program():
    if "nc" not in _program_cache:
        _program_cache["nc"] = _build_program()
    return _program_cache["nc"]


def kernel(**inputs) -> np.ndarray:
    B = FULL_B // N_CORES
    x_full = np.asarray(inputs["inputs"])
    assert x_full.shape == (FULL_B, S), x_full.shape
    x16 = np.ascontiguousarray(x_full.astype(np.int16))

    nc = _get_program()
    consts = _host_consts()
    in_maps = []
    for c in range(N_CORES):
        m = {"x": np.ascontiguousarray(x16[c * B:(c + 1) * B])}
        m.update(consts)
        in_maps.append(m)

    res = run_bass_kernel_spmd(nc, in_maps, core_ids=list(range(N_CORES)))
    ys = [np.asarray(res.results[c]["y"]) for c in range(N_CORES)]
    full = np.concatenate(ys, axis=0).astype(np.float32)
    # device bin order is (l, h); v = 32*h + l -> permute to v order
    full = full.reshape(FULL_B, NL, NH).transpose(0, 2, 1).reshape(FULL_B, V)
    return np.ascontiguousarray(full[:, 1:1100])
